# revision 45
# baseline (speedup 1.0000x reference)
"""Trainium2 Bass kernel for EnhancedOFTQKVLayer.

Computes out[b,s,o] = x[b,s,:] @ filt[o,:]^T + bias[o] where
filt = [Wq @ BD(cayley(q_R)); Wk @ BD(cayley(k_R)); Wv @ BD(cayley(v_R))]
(BD = block-diagonal, cayley(A) = (I-S) inv(I+S+eps I), S = 0.5(A-A^T)).

Distribution: data-parallel — batch b (8 rows) sharded one per NeuronCore;
attn_weight / bias / rotation blocks replicated. Each core:
  1. Cayley via SPD Newton-Schulz on P = (1+eps)^2 I - S^2 (iterates are
     polynomials in S^2, hence symmetric -> lhsT=operand works without
     transposes; periodic symmetrization kills roundoff-asymmetry growth).
     fp16 iterations with U = 2I - PX formed on the vector engine (8 PE
     matmuls per set-iteration, subtract on DVE, iterate-cast on ScalarE)
     + fp32 polish.
  2. W cast to a bf16 DRAM scratch by the SWDGE cast-DMA (gpsimd queue has
     nothing else), then pulled into SBUF *already transposed* by 48 big
     DMA-xbar loads; filtT = Q^T W^T with 48 N=512 matmuls. Zero compute-
     engine cost for the whole W path.
  3. Main matmul in bf16 (fp32 PSUM accumulation), o-group-outer loop so
     each PSUM bank is filled by one dense 8-matmul accumulation group and
     evicted (fused bias add on DVE) while later groups run. x row-tiles:
     HWDGE fp32 load -> ScalarE bf16 cast -> 8 SBUF->SBUF xbar transposes,
     pipelined one tile ahead of the matmul stream. One coalesced 1.5 MB
     output DMA per row-tile.
"""

import numpy as np

import concourse.bass as bass
import concourse.mybir as mybir
import concourse.tile as tile
from concourse import bacc
from concourse.bass import ts
from concourse.masks import make_identity
from concourse.bass_utils import run_bass_kernel_spmd

F32 = mybir.dt.float32
F16 = mybir.dt.float16
BF16 = mybir.dt.bfloat16

MAIN_DT = BF16           # dtype of the big matmul inputs (x, filtT)

HIDDEN = 1024
OUT_DIM = 3 * HIDDEN
SEQ = 4096
P = 128
NBLK = 8                 # 128-blocks per hidden
NROT = 24                # 3 * NBLK rotation blocks
EPS = 1e-6
N_CORES = 8

NSETS = 6                # Newton processes blocks in sets of 4
SETB = 4

# Newton-Schulz schedule (validated offline against the jax reference).
NEWTON_F16 = 7
NEWTON_F32 = 1
SYM_ITERS = {3, 5}       # symmetrize on these fp16 iterations
X0_A = 0.0152174         # X0 = aI + bP (degree-1 minimax init on [1, 260])
X0_B = -5.78922e-05

M_TILES = SEQ // P       # 32
O_TILES = OUT_DIM // 512  # 6
W_TILES = OUT_DIM // P   # 24
XPF = 3                  # x load prefetch depth in phase C


def build_body(ctx, tc):
    nc = tc.nc

    x = nc.dram_tensor("x", [SEQ, HIDDEN], F32, kind="ExternalInput").ap()
    w = nc.dram_tensor("w", [OUT_DIM, HIDDEN], F32, kind="ExternalInput").ap()
    bias = nc.dram_tensor("bias", [OUT_DIM], F32, kind="ExternalInput").ap()
    rmat = nc.dram_tensor("rmat", [NROT, P, P], F32, kind="ExternalInput").ap()
    out = nc.dram_tensor("out", [SEQ, OUT_DIM], F32, kind="ExternalOutput").ap()

    sub = nc.vector.tensor_sub
    add = nc.vector.tensor_add
    smul = nc.vector.tensor_scalar_mul
    cp = nc.vector.tensor_copy
    scp = nc.scalar.copy

    def bc(t):  # broadcast a [P, P] constant over a set's middle dim
        return t[:].unsqueeze(1).to_broadcast([P, SETB, P])

    # ---- persistent pools ----
    const = ctx.enter_context(tc.tile_pool(name="const", bufs=1))
    ftp = ctx.enter_context(tc.tile_pool(name="ftp", bufs=1))

    ident32 = const.tile([P, P], F32)
    make_identity(nc, ident32)
    eI2 = const.tile([P, P], F32)       # (1+eps)^2 I
    smul(eI2[:], ident32[:], float((1.0 + EPS) ** 2))
    eI12 = const.tile([P, P], F32)      # ((1+eps) + (1+eps)^2) I
    smul(eI12[:], ident32[:], float((1.0 + EPS) + (1.0 + EPS) ** 2))
    twoI = const.tile([P, P], F32)      # 2 I
    smul(twoI[:], ident32[:], 2.0)
    aI0 = const.tile([P, P], F16)       # X0_A * I  (Newton init)
    smul(aI0[:], ident32[:], float(X0_A))

    # bias broadcast (gpsimd queue: this, then only the W prepass DMAs)
    bias_bc = const.tile([P, OUT_DIM], MAIN_DT)
    with tc.tile_pool(name="biasld", bufs=1) as bl:
        brow = bl.tile([1, OUT_DIM], F32)
        nc.sync.dma_start(brow[:], bias.unsqueeze(0))
        cp(bias_bc[:1, :], brow[:])
    nc.gpsimd.partition_broadcast(bias_bc[:], bias_bc[:1, :])

    # filtT chunks: ft[k][og][c, o'] = filtT[k*128+c, og*512+o']
    ft = [[ftp.tile([P, 512], MAIN_DT, tag=f"ft{k}_{og}", name=f"ft{k}_{og}")
           for og in range(O_TILES)] for k in range(NBLK)]

    # ---- phase A+B scoped pools ----
    with (
        tc.tile_pool(name="nper", bufs=1) as nper,     # per-set persistents
        tc.tile_pool(name="ap", bufs=24) as apool,     # all rmat blocks
        tc.tile_pool(name="nx", bufs=1) as nxp,        # per-set iterates
        tc.tile_pool(name="nu", bufs=3) as nup,        # U temp
        tc.tile_pool(name="misc", bufs=1) as misc,
        tc.tile_pool(name="qpool", bufs=1) as qpool,
        tc.tile_pool(name="wld", bufs=4) as wld,
        tc.tile_pool(name="wtsp", bufs=4) as wtsp,
        tc.tile_pool(name="ps_g", bufs=6, space="PSUM") as ps_g,
        tc.tile_pool(name="ps_tp", bufs=2, space="PSUM") as ps_tp,
    ):
        # rmat: 24 contiguous [128,128] loads (clean DMA descriptors)
        asets = []
        for n in range(NROT):
            a = apool.tile([P, P], F32, tag="a", name=f"a{n}")
            nc.sync.dma_start(a[:], rmat[n])
            asets.append(a)

        # W: fp32 HWDGE row-tile loads, 4-deep ring
        wrows = []
        for ot in range(W_TILES):
            wrow = wld.tile([P, HIDDEN], F32, tag="wrow", name=f"wrow{ot}")
            nc.sync.dma_start(wrow[:], w[ts(ot, P), :])
            wrows.append(wrow)

        # W^T on the PE (fp32 transpose-mode; the PSUM->SBUF copy casts to
        # bf16), emitted og-by-og into Newton's PE-idle gaps.
        # wts[og][c, k, o'] = W[og*512 + j4*128 + o', k*128 + c]
        wts = [None] * O_TILES

        def emit_wT_rows(lo, hi):
            for ot in range(lo, min(hi, W_TILES)):
                og, j4 = ot // 4, ot % 4
                if wts[og] is None:
                    wts[og] = wtsp.tile([P, NBLK, 512], MAIN_DT, tag="wts",
                                        name=f"wts{og}")
                for kh in range(2):
                    tpg = ps_tp.tile([P, SETB, P], F32, tag="tp")
                    for k4 in range(SETB):
                        k = kh * SETB + k4
                        nc.tensor.transpose(tpg[:, k4, :],
                                            wrows[ot][:, ts(k, P)],
                                            ident32[:])
                    dst = wts[og][:, ts(kh, SETB), ts(j4, P)]
                    if (j4 + kh) % 2 == 0:
                        cp(dst, tpg[:])
                    else:
                        scp(dst, tpg[:])

        def emit_wT(og):
            emit_wT_rows(og * 4, og * 4 + 4)

        # ---------- Phase A: Newton-Cayley, 6 sets of 4 blocks ----------
        # S tiles first (set-major), then all S^2 matmuls back-to-back so
        # the PE never head-of-line blocks on a set's DVE chain.
        s_s, p32_s, p16_s, x_s, g_s = [], [], [], [], []
        for s in range(NSETS):
            tpg = ps_tp.tile([P, SETB, P], F32, tag="tp")
            for j in range(SETB):
                nc.tensor.transpose(tpg[:, j, :], asets[s * SETB + j][:],
                                    ident32[:])
            sset = nper.tile([P, SETB, P], F32, tag=f"s{s}", name=f"s{s}")
            for j in range(SETB):
                sub(sset[:, j, :], asets[s * SETB + j][:], tpg[:, j, :])
            smul(sset[:], sset[:], 0.5)                  # S
            s_s.append(sset)
        for s in range(NSETS):
            g = ps_g.tile([P, SETB, P], F32, tag="g")
            for j in range(SETB):                        # S^T @ S = -S^2
                nc.tensor.matmul(g[:, j, :], lhsT=s_s[s][:, j, :],
                                 rhs=s_s[s][:, j, :], start=True, stop=True)
            g_s.append(g)
        for s in range(NSETS):
            p32s = nper.tile([P, SETB, P], F32, tag=f"p32{s}", name=f"p32{s}")
            add(p32s[:], bc(eI2), g_s[s][:])             # P = (1+e)^2 I - S^2
            p16s = nper.tile([P, SETB, P], F16, tag=f"p16{s}", name=f"p16{s}")
            scp(p16s[:], p32s[:])
            xset = nxp.tile([P, SETB, P], F16, tag=f"x{s}", name=f"x{s}_init")
            smul(xset[:], p32s[:], float(X0_B))          # X0 = aI + bP
            add(xset[:], xset[:], bc(aI0))
            # fold B^T = eI12 + (2+e)S - P into the S tile now (Q only
            # needs B^T, and this keeps the Q phase off the critical chain)
            nc.vector.tensor_scalar(s_s[s][:], s_s[s][:], float(2.0 + EPS),
                                    None, mybir.AluOpType.mult)
            add(s_s[s][:], s_s[s][:], bc(eI12))
            sub(s_s[s][:], s_s[s][:], p32s[:])
            p32_s.append(p32s)
            p16_s.append(p16s)
            x_s.append(xset)

        emit_wT_rows(0, 2)
        for i in range(NEWTON_F16):
            do_sym = i in SYM_ITERS
            for s in range(NSETS):
                g1 = ps_g.tile([P, SETB, P], F32, tag="g")
                for j in range(SETB):
                    nc.tensor.matmul(g1[:, j, :], lhsT=p16_s[s][:, j, :],
                                     rhs=x_s[s][:, j, :], start=True, stop=True)
                u = nup.tile([P, SETB, P], F16, tag="u")
                sub(u[:], bc(twoI), g1[:])               # U = 2I - P X (DVE)
                g2 = ps_g.tile([P, SETB, P], F32, tag="g")
                for j in range(SETB):                    # X' = X U
                    nc.tensor.matmul(g2[:, j, :], lhsT=x_s[s][:, j, :],
                                     rhs=u[:, j, :], start=True, stop=True)
                xset = nxp.tile([P, SETB, P], F16, tag=f"x{s}",
                                name=f"x{s}_{i}")
                if not do_sym:
                    if s == 0:
                        cp(xset[:], g2[:])               # DVE
                    else:
                        scp(xset[:], g2[:])              # ScalarE
                else:
                    xc = misc.tile([P, SETB, P], F32, tag="xc")
                    cp(xc[:], g2[:])
                    tpg = ps_tp.tile([P, SETB, P], F32, tag="tp")
                    for j in range(SETB):
                        nc.tensor.transpose(tpg[:, j, :], xc[:, j, :],
                                            ident32[:])
                    add(xc[:], xc[:], tpg[:])
                    nc.scalar.activation(xset[:], xc[:],
                                         mybir.ActivationFunctionType.Copy,
                                         scale=0.5)
                x_s[s] = xset
            # 2 W row-tiles per iteration (og0-3; og4/5 follow in phase B)
            emit_wT_rows(2 + 2 * i, 4 + 2 * i)

        xf_s = []
        for s in range(NSETS):
            xf = nxp.tile([P, SETB, P], F32, tag=f"xf{s}", name=f"xf{s}_init")
            if s % 2 == 0:
                cp(xf[:], x_s[s][:])
            else:
                scp(xf[:], x_s[s][:])
            xf_s.append(xf)
        for i in range(NEWTON_F32):
            g1s = []
            for s in range(NSETS):
                g1 = ps_g.tile([P, SETB, P], F32, tag="g")
                for j in range(SETB):
                    nc.tensor.matmul(g1[:, j, :], lhsT=p32_s[s][:, j, :],
                                     rhs=xf_s[s][:, j, :], start=True,
                                     stop=True)
                g1s.append(g1)
            g2s = []
            for s in range(NSETS):
                uf = misc.tile([P, SETB, P], F32, tag="uf")
                sub(uf[:], bc(twoI), g1s[s][:])
                g2 = ps_g.tile([P, SETB, P], F32, tag="g")
                for j in range(SETB):
                    nc.tensor.matmul(g2[:, j, :], lhsT=xf_s[s][:, j, :],
                                     rhs=uf[:, j, :], start=True, stop=True)
                g2s.append(g2)
            for s in range(NSETS):
                xf = nxp.tile([P, SETB, P], F32, tag=f"xf{s}",
                              name=f"xf{s}_{i}")
                if s % 2 == 0:
                    cp(xf[:], g2s[s][:])
                else:
                    scp(xf[:], g2s[s][:])
                xf_s[s] = xf

        # Q = B @ X with B^T = eI12 + (2+e)S - P (pre-folded into s_s)
        qg_s, q_s = [], []
        for s in range(NSETS):
            g = ps_g.tile([P, SETB, P], F32, tag="g")
            for j in range(SETB):
                nc.tensor.matmul(g[:, j, :], lhsT=s_s[s][:, j, :],
                                 rhs=xf_s[s][:, j, :], start=True, stop=True)
            qg_s.append(g)
        for s in range(NSETS):
            qset = qpool.tile([P, SETB, P], MAIN_DT, tag=f"q{s}", name=f"q{s}")
            if s % 2 == 0:
                cp(qset[:], qg_s[s][:])
            else:
                scp(qset[:], qg_s[s][:])
            q_s.append(qset)

        def q_lhsT(n):
            return q_s[n // SETB][:, n % SETB, :]

        # ---------- Phase B: filtT = Q^T W^T ----------
        # og4/og5 W^T slots reuse og0/og1's ring entries, so they are
        # emitted only after the ft matmuls that consume og0/og1.
        for og in range(O_TILES):
            if og < 2:
                emit_wT(og + 4)
            part = og // 2             # which of q/k/v this group belongs to
            for k in range(NBLK):
                fg = ps_g.tile([P, 512], F32, tag="g")
                nc.tensor.matmul(fg[:], lhsT=q_lhsT(part * NBLK + k),
                                 rhs=wts[og][:, k, :], start=True, stop=True)
                if k % 2 == 0:
                    cp(ft[k][og][:], fg[:])
                else:
                    scp(ft[k][og][:], fg[:])

    # ---------- Phase C: main matmul, o-group-outer accumulation ----------
    with (
        tc.tile_pool(name="obp", bufs=2) as obp,
        tc.tile_pool(name="xrp", bufs=XPF) as xrp,
        tc.tile_pool(name="xtp", bufs=2) as xtp,
        tc.tile_pool(name="ps_out", bufs=6, space="PSUM") as ps_out,
        tc.tile_pool(name="ps_xt", bufs=1, space="PSUM") as ps_xt,
    ):
        def emit_xload(mt):
            xr = xrp.tile([P, HIDDEN], F32, tag="xr", name=f"xr{mt}")
            nc.sync.dma_start(xr[:], x[ts(mt, P), :])
            return xr

        def emit_xt(mt, xr):
            # PE fp32 transposes (transpose-mode, ~80ns each); the
            # PSUM->SBUF copy does the bf16 cast for free on ScalarE.
            tpg = ps_xt.tile([P, NBLK, P], F32, tag="xtp", name=f"xtp{mt}")
            for k in range(NBLK):
                nc.tensor.transpose(tpg[:, k, :], xr[:, ts(k, P)], ident32[:])
            xt = xtp.tile([P, NBLK, P], MAIN_DT, tag="xt", name=f"xt{mt}")
            scp(xt[:], tpg[:])
            return xt

        xr_ring = [emit_xload(mt) for mt in range(XPF)]
        xt_cur = emit_xt(0, xr_ring[0])
        for mt in range(M_TILES):
            last = mt == M_TILES - 1
            if mt + XPF < M_TILES:
                xr_ring.append(emit_xload(mt + XPF))
            ob = obp.tile([P, OUT_DIM], F32, tag="ob", name=f"ob{mt}")
            xt_nxt = None
            for o in range(O_TILES):
                po = ps_out.tile([P, 512], F32, tag="po", name=f"po{mt}_{o}")
                for k in range(NBLK):
                    nc.tensor.matmul(po[:], lhsT=xt_cur[:, k, :],
                                     rhs=ft[k][o][:],
                                     start=(k == 0), stop=(k == NBLK - 1))
                add(ob[:, ts(o, 512)], po[:], bias_bc[:, ts(o, 512)])
                if last:   # drain per o-group to shorten the tail
                    nc.sync.dma_start(out[ts(mt, P), ts(o, 512)],
                                      ob[:, ts(o, 512)])
                if o == 1 and not last:
                    xt_nxt = emit_xt(mt + 1, xr_ring[mt + 1])
            if not last:
                xt_cur = xt_nxt
                nc.sync.dma_start(out[ts(mt, P), :], ob[:])


def build():
    if "nc" in _CACHE:
        return _CACHE["nc"]
    import contextlib

    nc = bacc.Bacc("TRN2", target_bir_lowering=False, debug=False)
    with tile.TileContext(nc) as tc:
        with contextlib.ExitStack() as ctx:
            build_body(ctx, tc)
    nc.compile()
    _CACHE["nc"] = nc
    return nc


_CACHE = {}


def make_in_maps(attn_weight, bias, x, q_R, k_R, v_R):
    rmat = np.ascontiguousarray(
        np.concatenate([q_R, k_R, v_R], axis=0), dtype=np.float32)
    w = np.ascontiguousarray(attn_weight, dtype=np.float32)
    b = np.ascontiguousarray(bias, dtype=np.float32)
    return [
        {"x": np.ascontiguousarray(x[c], dtype=np.float32),
         "w": w, "bias": b, "rmat": rmat}
        for c in range(N_CORES)
    ]


def kernel(attn_weight, bias, x, q_R, k_R, v_R, **run_kwargs):
    nc = build()
    in_maps = make_in_maps(attn_weight, bias, x, q_R, k_R, v_R)
    res = run_bass_kernel_spmd(nc, in_maps, core_ids=list(range(N_CORES)),
                               **run_kwargs)
    out = np.stack([res.results[c]["out"] for c in range(N_CORES)], axis=0)
    _CACHE["last_results"] = res
    return out
